# revision 10
# baseline (speedup 1.0000x reference)
"""Trainium2 Bass kernel for nn_MultiHeadAttention (B=8, S=1024, D=1024, h=16).

Sharding: pure data-parallel over batch — each of the 8 NeuronCores computes
the full MHA for one batch element. No collectives.

Host side pre-casts Q/K/V and the four weight matrices to bf16 (the PE
operands are bf16 regardless, so this loses nothing) — halving input HBM
traffic and removing every on-chip f32->bf16 cast.

Per-core pipeline (bf16 matmul operands, fp32 PSUM accumulation):
  1. Inputs arrive feature-major via DMA XBAR transpose straight from HBM
     (dma_start_transpose, bf16): no PE transposes, no PSUM staging, no
     eviction traffic. Each weight matrix is a single 2MB DMA into one
     [128, 8x1024] SBUF tile.
  2. Projections: stationary = weight block.
       Q/K: feature-major relu outputs (ACT fuses bias+relu per-partition);
       V: row-major, evicted head-major into "Vaug" tiles (per head 65 cols
       = 64 V-features + a ones column that accumulates the softmax
       denominator in the PV matmul). V/O biases are added by a K=1 matmul
       (ones-row (x) bias-row outer product accumulated into PSUM), so the
       eviction is a single DVE tensor_scalar_max (relu) — Scalar stays
       free for the exp stream.
  3. Attention per head-PAIR: even head on PE rows 0-63, odd head on rows
     64-127. Scores are kept transposed (keys on partitions, queries free):
       P_T = exp(S_T / 32)  (ACT; scores are O(0.3): no max-subtraction)
     Softmax division: the denominator row is DMA-spread to [64, 8] (DVE
     reciprocal is ~6.5ns/elem per partition-stream, so it must be spread),
     reciprocal'd, gathered back to a bf16 row by a casting gpsimd DMA,
     broadcast across partitions by a K=1 PE outer product, then one DVE
     multiply. No DRAM round trips.
  4. Output projection relu(Ot @ WO + bO): contraction blocks 0-6 of every
     seq-block run while the last heads' divisions drain; block 7 +
     eviction + store land per seq-block right after.
"""
import os
from contextlib import ExitStack

import ml_dtypes
import numpy as np

import concourse.bass as bass
import concourse.tile as tile
from concourse import mybir
from concourse.bass_utils import run_bass_kernel_spmd

f32 = mybir.dt.float32
bf16 = mybir.dt.bfloat16
AF = mybir.ActivationFunctionType

S = 1024
D = 1024
H = 16
DK = 64
P = 128
NB = D // P  # 8 blocks
QC = 512
N_CORES = 8


def _split_wide_waits(nc, max_waits=1):
    """This walrus build rejects instructions carrying more than one
    semaphore wait; move excess waits onto NoOp carriers inserted before
    the offending instruction on the same engine."""
    for bb in nc.m.functions[0].blocks:
        idx = 0
        while idx < len(bb.instructions):
            ins = bb.instructions[idx]
            si = ins.sync_info
            if si is not None and si.on_wait and len(si.on_wait) > max_waits:
                waits = list(si.on_wait)
                rest, keep = waits[:-max_waits], waits[-max_waits:]
                for j in range(0, len(rest), max_waits):
                    nop = mybir.InstNoOp(
                        name=f"I-waitsplit-{nc.next_id()}",
                        engine=ins.engine,
                        ins=[],
                        outs=[],
                    )
                    nop.sync_info = mybir.SyncInfo(
                        on_wait=rest[j : j + max_waits], on_update=[]
                    )
                    nc.register_instruction(nop)
                    bb.instructions.insert(idx, nop)
                    idx += 1
                ins.sync_info = mybir.SyncInfo(
                    on_wait=keep, on_update=list(si.on_update)
                )
            idx += 1


def _build_nc(with_bv: bool, with_bo: bool):
    nc = bass.Bass("TRN2", target_bir_lowering=False, debug=False, num_devices=1)

    Qd = nc.dram_tensor("Q", [S, D], bf16, kind="ExternalInput").ap()
    Kd = nc.dram_tensor("K", [S, D], bf16, kind="ExternalInput").ap()
    Vd = nc.dram_tensor("V", [S, D], bf16, kind="ExternalInput").ap()
    WQd = nc.dram_tensor("WQ", [D, D], bf16, kind="ExternalInput").ap()
    WKd = nc.dram_tensor("WK", [D, D], bf16, kind="ExternalInput").ap()
    WVd = nc.dram_tensor("WV", [D, D], bf16, kind="ExternalInput").ap()
    WOd = nc.dram_tensor("WO", [D, D], bf16, kind="ExternalInput").ap()
    bQd = nc.dram_tensor("bQ", [D], f32, kind="ExternalInput").ap()
    bKd = nc.dram_tensor("bK", [D], f32, kind="ExternalInput").ap()
    bVd = nc.dram_tensor("bV", [D], f32, kind="ExternalInput").ap()
    bOd = nc.dram_tensor("bO", [D], f32, kind="ExternalInput").ap()
    outd = nc.dram_tensor("out", [S, D], f32, kind="ExternalOutput").ap()

    with tile.TileContext(nc) as tc, ExitStack() as ctx:
        sb = ctx.enter_context(tc.tile_pool(name="sb", bufs=1))
        ps = ctx.enter_context(tc.tile_pool(name="ps", bufs=1, space="PSUM"))
        dramp = ctx.enter_context(tc.tile_pool(name="dram", bufs=1, space="DRAM"))

        # ---- constants -------------------------------------------------
        onesb = sb.tile([1, P], bf16, tag="onesb", name="onesb")
        nc.vector.memset(onesb, 1.0)
        bqk = sb.tile([P, 2 * NB], f32, tag="bqk", name="bqk")
        nc.sync.dma_start(bqk[:, 0:NB], bQd.rearrange("(db p) -> p db", p=P))
        nc.sync.dma_start(bqk[:, NB : 2 * NB], bKd.rearrange("(db p) -> p db", p=P))
        if with_bv:
            bvs = sb.tile([1, D], f32, tag="brows", bufs=2, name="bvs")
            nc.sync.dma_start(bvs, bVd[None, :])
            bv_row = sb.tile([1, D], bf16, tag="browb", bufs=2, name="bv_row")
            nc.vector.tensor_copy(bv_row, bvs)
        if with_bo:
            bos = sb.tile([1, D], f32, tag="brows", bufs=2, name="bos")
            nc.sync.dma_start(bos, bOd[None, :])
            bo_row = sb.tile([1, D], bf16, tag="browb", bufs=2, name="bo_row")
            nc.vector.tensor_copy(bo_row, bos)

        def wload(Wd):
            """One 2MB DMA: whole weight matrix as [p, kb, m] with
            wb[p, kb, m] = W[kb*128 + p, m]."""
            wb = sb.tile([P, NB, D], bf16, tag="wb", bufs=2, name="wb")
            nc.sync.dma_start(wb, Wd.rearrange("(kb p) m -> p kb m", p=P))
            return wb

        def load_transposed(Xd):
            """HBM row-major bf16 -> feature-major tiles xt[db] (128 x 1024)
            via the DMA XBAR transpose. No PE/PSUM involvement."""
            xt = [
                sb.tile([P, S], bf16, tag="xt", bufs=NB, name=f"xt{i}")
                for i in range(NB)
            ]
            for db in range(NB):
                nc.sync.dma_start_transpose(xt[db], Xd[:, db * P : (db + 1) * P])
            return xt

        def proj_feature_major(xt, wb, bias_base, out_tag):
            """xpt[db] = relu(W[:,db-block].T @ xt + b[db-block]) -> bf16."""
            xpt = [
                sb.tile([P, S], bf16, tag=out_tag, bufs=NB, name=f"{out_tag}{i}")
                for i in range(NB)
            ]
            for db in range(NB):
                acc = ps.tile([P, 2, QC], f32, tag="big", bufs=2, name="acc")
                co = db * P
                for kb in range(NB):
                    wt = wb[:, kb, co : co + P]
                    first, last = kb == 0, kb == NB - 1
                    nc.tensor.matmul(
                        acc[:, 0, :], wt, xt[kb][:, 0:QC],
                        start=first, stop=last,
                    )
                    nc.tensor.matmul(
                        acc[:, 1, :], wt, xt[kb][:, QC:S],
                        start=first, stop=last,
                    )
                nc.scalar.activation(
                    xpt[db].rearrange("p (c q) -> p c q", c=2),
                    acc,
                    AF.Relu,
                    bias=bqk[:, bias_base + db : bias_base + db + 1],
                )
            return xpt

        # ---- Q / K ------------------------------------------------------
        with nc.named_scope("q_prep"):
            xt = load_transposed(Qd)
            wq = wload(WQd)
        with nc.named_scope("q_proj"):
            qpt = proj_feature_major(xt, wq, 0, "qpt")
        with nc.named_scope("k_prep"):
            xt = load_transposed(Kd)
            wk = wload(WKd)
        with nc.named_scope("k_proj"):
            kpt = proj_feature_major(xt, wk, NB, "kpt")

        # early: scores+exp for head pair 0 — its ACT exp work overlaps the
        # V prep/projection below, which otherwise leaves ScalarE idle
        def emit_scores_unit(d, pv_gen):
            ptA = sb.tile([P, NB, 2, QC], bf16, tag="pt", bufs=4, name="ptA")
            ptB = sb.tile([P, NB, 2, QC], bf16, tag="pt", bufs=4, name="ptB")
            for kb in range(NB):
                ksl = slice(kb * P, (kb + 1) * P)
                spA = ps.tile([P, 2, QC], f32, tag="big", bufs=2, name="spA")
                spB = ps.tile([P, 2, QC], f32, tag="big", bufs=2, name="spB")
                for qc in range(2):
                    qsl = slice(qc * QC, (qc + 1) * QC)
                    nc.tensor.matmul(
                        spA[:, qc, :], kpt[d][0:DK, ksl], qpt[d][0:DK, qsl],
                        start=True, stop=True,
                    )
                for qc in range(2):
                    qsl = slice(qc * QC, (qc + 1) * QC)
                    nc.tensor.matmul(
                        spB[:, qc, :], kpt[d][DK:P, ksl], qpt[d][DK:P, qsl],
                        start=True, stop=True,
                    )
                nc.scalar.activation(
                    ptA[:, kb, :, :], spA, AF.Exp, scale=0.03125
                )
                nc.scalar.activation(
                    ptB[:, kb, :, :], spB, AF.Exp, scale=0.03125
                )
                if pv_gen is not None:
                    next(pv_gen, None)
            return ptA, ptB

        early_unit = emit_scores_unit(0, None)

        # ---- V ----------------------------------------------------------
        with nc.named_scope("v_prep"):
            vt = load_transposed(Vd)
            wv = wload(WVd)
            wo = wload(WOd)
        with nc.named_scope("v_proj"):
            vaug = [
                sb.tile([P, H * 65], bf16, tag="vaug", bufs=NB, name=f"vaug{i}")
                for i in range(NB)
            ]
            for sblk in range(NB):
                nc.vector.memset(
                    vaug[sblk].rearrange("p (h c) -> p h c", c=65)[:, :, 64:65],
                    1.0,
                )
            for sblk in range(NB):
                acc = [
                    ps.tile([P, QC], f32, tag="vp", bufs=2, name="vacc")
                    for _ in range(2)
                ]
                if with_bv:
                    for c in range(2):
                        nc.tensor.matmul(
                            acc[c],
                            onesb[0:1, 0:P],
                            bv_row[0:1, c * QC : (c + 1) * QC],
                            start=True, stop=False,
                        )
                for kb in range(NB):
                    for c in range(2):
                        nc.tensor.matmul(
                            acc[c],
                            vt[kb][:, sblk * P : (sblk + 1) * P],
                            wv[:, kb, c * QC : (c + 1) * QC],
                            start=(kb == 0 and not with_bv),
                            stop=(kb == NB - 1),
                        )
                for c in range(2):
                    dst = vaug[sblk].rearrange("p (h c) -> p h c", c=65)[
                        :, c * 8 : (c + 1) * 8, 0:64
                    ]
                    nc.vector.tensor_scalar_max(
                        dst, acc[c].rearrange("p (h c) -> p h c", c=64), 0.0
                    )

        # ---- attention --------------------------------------------------
        ot = [
            sb.tile([P, S], bf16, tag="ot", bufs=NB, name=f"ot{i}")
            for i in range(NB)
        ]

        def emit_pv_tail(h, vp):
            """Softmax division: spread denom row -> DVE reciprocal ->
            casting gather -> K=1 PE partition-broadcast -> DVE multiply."""
            dbq, off = h // 2, (h % 2) * DK
            for qc in range(2):
                qsl = slice(qc * QC, (qc + 1) * QC)
                stage = sb.tile([65, QC], f32, tag="stage", bufs=4, name="stage")
                nc.vector.tensor_copy(stage, vp[qc][0:65, :])
                # SBUF can't scatter one partition's row across partitions
                # (illegal partition step), so bounce the denom row off DRAM
                scr = dramp.tile([1, QC], f32, tag="scr", bufs=4, name="scr")
                nc.sync.dma_start(scr, stage[64:65, :])
                rsp = sb.tile([DK, NB], f32, tag="rsp", bufs=4, name="rsp")
                nc.sync.dma_start(
                    rsp, scr.rearrange("o (a b) -> a (o b)", a=DK)
                )
                nc.vector.reciprocal(rsp, rsp)
                rrow = sb.tile([1, QC], bf16, tag="rrow", bufs=4, name="rrow")
                # [64, 8] -> [1, 512] partition-major gather (+ f32->bf16
                # cast, hence gpsimd); AP balancing maps (a, b) -> a*8+b
                nc.gpsimd.dma_start(rrow.rearrange("o (a b) -> o a b", a=DK), rsp)
                bc = ps.tile([DK, QC], f32, tag="bc", bufs=2, name="bc")
                nc.tensor.matmul(bc, onesb[0:1, 0:DK], rrow, start=True, stop=True)
                if off == 0:
                    nc.vector.tensor_mul(ot[dbq][0:DK, qsl], stage[0:DK, :], bc)
                else:
                    tmp = sb.tile([DK, QC], bf16, tag="tmp", bufs=2, name="tmp")
                    nc.vector.tensor_mul(tmp, stage[0:DK, :], bc)
                    nc.gpsimd.dma_start(ot[dbq][DK:P, qsl], tmp)

        def gen_pv_pair(d, ptA, ptB):
            """PV + softmax division for head pair (2d, 2d+1), both q-chunks,
            yielded in 8 groups of 4 matmuls for interleaving."""
            for hl, ptX in ((0, ptA), (1, ptB)):
                h = 2 * d + hl
                vp = [
                    ps.tile([P, QC], f32, tag="vp", bufs=2, name="vpacc")
                    for _ in range(2)
                ]
                for g in range(4):
                    for kb in (2 * g, 2 * g + 1):
                        for qc in range(2):
                            nc.tensor.matmul(
                                vp[qc][0:65, :],
                                vaug[kb][:, h * 65 : (h + 1) * 65],
                                ptX[:, kb, qc, :],
                                start=(kb == 0),
                                stop=(kb == NB - 1),
                            )
                    yield
                emit_pv_tail(h, vp)

        def emit_oproj():
            """O-projection with contraction block 7 deferred: blocks 0-6 of
            all seq-blocks have no dependency on the last head pair, so the
            PE starts them while the last softmax divisions drain on Vector;
            block 7 + eviction land per seq-block as the PSUM ring allows."""
            accs = [None] * NB

            def finish(s):
                bigacc = accs[s]
                for c in range(2):
                    nc.tensor.matmul(
                        bigacc[:, c, :],
                        ot[NB - 1][:, s * P : (s + 1) * P],
                        wo[:, NB - 1, c * QC : (c + 1) * QC],
                        start=False, stop=True,
                    )
                for c in range(2):
                    o = sb.tile([P, QC], f32, tag="obuf", bufs=3, name="obuf")
                    nc.vector.tensor_scalar_max(o, bigacc[:, c, :], 0.0)
                    nc.sync.dma_start(
                        outd[s * P : (s + 1) * P, c * QC : (c + 1) * QC], o
                    )
                accs[s] = None

            for s in range(NB):
                bigacc = ps.tile([P, 2, QC], f32, tag="big", bufs=2, name="oacc")
                accs[s] = bigacc
                if with_bo:
                    for c in range(2):
                        nc.tensor.matmul(
                            bigacc[:, c, :],
                            onesb[0:1, 0:P],
                            bo_row[0:1, c * QC : (c + 1) * QC],
                            start=True, stop=False,
                        )
                for db in range(NB - 1):
                    for c in range(2):
                        nc.tensor.matmul(
                            bigacc[:, c, :],
                            ot[db][:, s * P : (s + 1) * P],
                            wo[:, db, c * QC : (c + 1) * QC],
                            start=(db == 0 and not with_bo),
                            stop=False,
                        )
                if s >= 1:
                    finish(s - 1)
            finish(NB - 1)

        with nc.named_scope("attention"):
            # software-pipelined over head pairs; the previous pair's PV
            # matmul groups ride inside the current scores unit. Pair 0's
            # scores were hoisted before the V phase.
            pend = early_unit
            for d in range(1, NB):
                g = gen_pv_pair(d - 1, *pend)
                pend = emit_scores_unit(d, g)
                for _ in g:
                    pass

        # ---- output projection --------------------------------------
        with nc.named_scope("o_proj"):
            for _ in gen_pv_pair(NB - 1, *pend):
                pass
            emit_oproj()

    _split_wide_waits(nc)
    return nc


_NC_CACHE = {}


def kernel(Q, K, V, WQ, bQ, WK, bK, WV, bV, WO, bO, h):
    bfl = ml_dtypes.bfloat16
    Q, K, V = (np.ascontiguousarray(np.asarray(x, np.float32).astype(bfl))
               for x in (Q, K, V))
    WQ, WK, WV, WO = (
        np.ascontiguousarray(np.asarray(x, np.float32).astype(bfl))
        for x in (WQ, WK, WV, WO)
    )
    bQ, bK, bV, bO = (
        np.ascontiguousarray(np.asarray(x, np.float32)) for x in (bQ, bK, bV, bO)
    )
    h = int(np.asarray(h))
    assert h == H, f"kernel specialized for h=16, got {h}"
    B = Q.shape[0]
    assert Q.shape == (B, S, D) and B == N_CORES

    key = (bool(np.any(bV)), bool(np.any(bO)))
    if key not in _NC_CACHE:
        _NC_CACHE[key] = _build_nc(*key)
    nc = _NC_CACHE[key]

    in_maps = [
        {
            "Q": Q[b], "K": K[b], "V": V[b],
            "WQ": WQ, "WK": WK, "WV": WV, "WO": WO,
            "bQ": bQ, "bK": bK, "bV": bV, "bO": bO,
        }
        for b in range(B)
    ]
    trace = os.environ.get("BASS_MHA_TRACE") == "1"
    res = run_bass_kernel_spmd(
        nc, in_maps, core_ids=list(range(N_CORES)), trace=trace
    )
    if trace:
        kernel.last_results = res
    return np.stack([res.results[b]["out"] for b in range(B)], axis=0)


# revision 13
# speedup vs baseline: 1.0419x; 1.0419x over previous
"""Trainium2 Bass kernel for nn_MultiHeadAttention (B=8, S=1024, D=1024, h=16).

Sharding: pure data-parallel over batch — each of the 8 NeuronCores computes
the full MHA for one batch element. No collectives.

Host side pre-casts Q/K/V and the four weight matrices to bf16 (the PE
operands are bf16 regardless, so this loses nothing) — halving input HBM
traffic and removing every on-chip f32->bf16 cast.

Per-core pipeline (bf16 matmul operands, fp32 PSUM accumulation):
  1. Inputs arrive feature-major via DMA XBAR transpose straight from HBM
     (dma_start_transpose, bf16): no PE transposes, no PSUM staging, no
     eviction traffic. Each weight matrix is a single 2MB DMA into one
     [128, 8x1024] SBUF tile.
  2. Projections: stationary = weight block.
       Q/K: feature-major relu outputs (ACT fuses bias+relu per-partition);
       V: row-major, evicted head-major into "Vaug" tiles (per head 65 cols
       = 64 V-features + a ones column that accumulates the softmax
       denominator in the PV matmul). V/O biases are added by a K=1 matmul
       (ones-row (x) bias-row outer product accumulated into PSUM), so the
       eviction is a single DVE tensor_scalar_max (relu) — Scalar stays
       free for the exp stream.
  3. Attention per head-PAIR: even head on PE rows 0-63, odd head on rows
     64-127. Scores are kept transposed (keys on partitions, queries free):
       P_T = exp(S_T / 32)  (ACT; scores are O(0.3): no max-subtraction)
     Softmax division: the denominator row is DMA-spread to [64, 8] (DVE
     reciprocal is ~6.5ns/elem per partition-stream, so it must be spread),
     reciprocal'd, gathered back to a bf16 row by a casting gpsimd DMA,
     broadcast across partitions by a K=1 PE outer product, then one DVE
     multiply. No DRAM round trips.
  4. Output projection relu(Ot @ WO + bO): contraction blocks 0-6 of every
     seq-block run while the last heads' divisions drain; block 7 +
     eviction + store land per seq-block right after.
"""
import os
from contextlib import ExitStack

import ml_dtypes
import numpy as np

import concourse.bass as bass
import concourse.tile as tile
from concourse import mybir
from concourse.bass_utils import run_bass_kernel_spmd

f32 = mybir.dt.float32
bf16 = mybir.dt.bfloat16
AF = mybir.ActivationFunctionType

S = 1024
D = 1024
H = 16
DK = 64
P = 128
NB = D // P  # 8 blocks
QC = 512
N_CORES = 8


def _split_wide_waits(nc, max_waits=1):
    """This walrus build rejects instructions carrying more than one
    semaphore wait; move excess waits onto NoOp carriers inserted before
    the offending instruction on the same engine."""
    for bb in nc.m.functions[0].blocks:
        idx = 0
        while idx < len(bb.instructions):
            ins = bb.instructions[idx]
            si = ins.sync_info
            if si is not None and si.on_wait and len(si.on_wait) > max_waits:
                waits = list(si.on_wait)
                rest, keep = waits[:-max_waits], waits[-max_waits:]
                for j in range(0, len(rest), max_waits):
                    nop = mybir.InstNoOp(
                        name=f"I-waitsplit-{nc.next_id()}",
                        engine=ins.engine,
                        ins=[],
                        outs=[],
                    )
                    nop.sync_info = mybir.SyncInfo(
                        on_wait=rest[j : j + max_waits], on_update=[]
                    )
                    nc.register_instruction(nop)
                    bb.instructions.insert(idx, nop)
                    idx += 1
                ins.sync_info = mybir.SyncInfo(
                    on_wait=keep, on_update=list(si.on_update)
                )
            idx += 1


def _build_nc(with_bv: bool, with_bo: bool):
    nc = bass.Bass("TRN2", target_bir_lowering=False, debug=False, num_devices=1)

    Qd = nc.dram_tensor("Q", [S, D], bf16, kind="ExternalInput").ap()
    Kd = nc.dram_tensor("K", [S, D], bf16, kind="ExternalInput").ap()
    Vd = nc.dram_tensor("V", [S, D], bf16, kind="ExternalInput").ap()
    WQd = nc.dram_tensor("WQ", [D, D], bf16, kind="ExternalInput").ap()
    WKd = nc.dram_tensor("WK", [D, D], bf16, kind="ExternalInput").ap()
    WVd = nc.dram_tensor("WV", [D, D], bf16, kind="ExternalInput").ap()
    WOd = nc.dram_tensor("WO", [D, D], bf16, kind="ExternalInput").ap()
    bQd = nc.dram_tensor("bQ", [D], f32, kind="ExternalInput").ap()
    bKd = nc.dram_tensor("bK", [D], f32, kind="ExternalInput").ap()
    bVd = nc.dram_tensor("bV", [D], f32, kind="ExternalInput").ap()
    bOd = nc.dram_tensor("bO", [D], f32, kind="ExternalInput").ap()
    outd = nc.dram_tensor("out", [S, D], f32, kind="ExternalOutput").ap()

    with tile.TileContext(nc) as tc, ExitStack() as ctx:
        sb = ctx.enter_context(tc.tile_pool(name="sb", bufs=1))
        ps = ctx.enter_context(tc.tile_pool(name="ps", bufs=1, space="PSUM"))
        dramp = ctx.enter_context(tc.tile_pool(name="dram", bufs=1, space="DRAM"))

        # ---- constants -------------------------------------------------
        onesb = sb.tile([1, P], bf16, tag="onesb", name="onesb")
        nc.vector.memset(onesb, 1.0)
        bqk = sb.tile([P, 2 * NB], f32, tag="bqk", name="bqk")
        nc.sync.dma_start(bqk[:, 0:NB], bQd.rearrange("(db p) -> p db", p=P))
        nc.sync.dma_start(bqk[:, NB : 2 * NB], bKd.rearrange("(db p) -> p db", p=P))
        if with_bv:
            bvs = sb.tile([1, D], f32, tag="brows", bufs=2, name="bvs")
            nc.sync.dma_start(bvs, bVd[None, :])
            bv_row = sb.tile([1, D], bf16, tag="browb", bufs=2, name="bv_row")
            nc.vector.tensor_copy(bv_row, bvs)
        if with_bo:
            bos = sb.tile([1, D], f32, tag="brows", bufs=2, name="bos")
            nc.sync.dma_start(bos, bOd[None, :])
            bo_row = sb.tile([1, D], bf16, tag="browb", bufs=2, name="bo_row")
            nc.vector.tensor_copy(bo_row, bos)

        def wload(Wd):
            """One 2MB DMA: whole weight matrix as [p, kb, m] with
            wb[p, kb, m] = W[kb*128 + p, m]."""
            wb = sb.tile([P, NB, D], bf16, tag="wb", bufs=2, name="wb")
            nc.sync.dma_start(wb, Wd.rearrange("(kb p) m -> p kb m", p=P))
            return wb

        def load_transposed(Xd):
            """HBM row-major bf16 -> feature-major tiles xt[db] (128 x 1024)
            via the DMA XBAR transpose (issued from the Scalar HWDGE queue;
            each occupies the issuing engine ~1.3us, and Sync carries the
            weight streams). bufs=12 lets the next tensor's first transposes
            prefetch while the previous projection still reads its tiles."""
            xt = [
                sb.tile([P, S], bf16, tag="xt", bufs=12, name=f"xt{i}")
                for i in range(NB)
            ]
            for db in range(NB):
                nc.scalar.dma_start_transpose(xt[db], Xd[:, db * P : (db + 1) * P])
            return xt

        def proj_feature_major(xt, wb, bias_base, out_tag):
            """xpt[db] = relu(W[:,db-block].T @ xt + b[db-block]) -> bf16."""
            xpt = [
                sb.tile([P, S], bf16, tag=out_tag, bufs=NB, name=f"{out_tag}{i}")
                for i in range(NB)
            ]
            for db in range(NB):
                acc = ps.tile([P, 2, QC], f32, tag="big", bufs=2, name="acc")
                co = db * P
                for kb in range(NB):
                    wt = wb[:, kb, co : co + P]
                    first, last = kb == 0, kb == NB - 1
                    nc.tensor.matmul(
                        acc[:, 0, :], wt, xt[kb][:, 0:QC],
                        start=first, stop=last,
                    )
                    nc.tensor.matmul(
                        acc[:, 1, :], wt, xt[kb][:, QC:S],
                        start=first, stop=last,
                    )
                nc.scalar.activation(
                    xpt[db].rearrange("p (c q) -> p c q", c=2),
                    acc,
                    AF.Relu,
                    bias=bqk[:, bias_base + db : bias_base + db + 1],
                )
            return xpt

        # ---- Q / K ------------------------------------------------------
        with nc.named_scope("q_prep"):
            xt = load_transposed(Qd)
            wq = wload(WQd)
            wk = wload(WKd)
        with nc.named_scope("q_proj"):
            qpt = proj_feature_major(xt, wq, 0, "qpt")
        with nc.named_scope("k_prep"):
            xt = load_transposed(Kd)
        with nc.named_scope("k_proj"):
            kpt = proj_feature_major(xt, wk, NB, "kpt")

        # early: scores+exp for head pair 0 — its ACT exp work overlaps the
        # V prep/projection below, which otherwise leaves ScalarE idle
        def emit_scores_unit(d, pv_gen):
            ptA = sb.tile([P, NB, 2, QC], bf16, tag="pt", bufs=4, name="ptA")
            ptB = sb.tile([P, NB, 2, QC], bf16, tag="pt", bufs=4, name="ptB")
            for kb in range(NB):
                ksl = slice(kb * P, (kb + 1) * P)
                spA = ps.tile([P, 2, QC], f32, tag="big", bufs=2, name="spA")
                spB = ps.tile([P, 2, QC], f32, tag="big", bufs=2, name="spB")
                for qc in range(2):
                    qsl = slice(qc * QC, (qc + 1) * QC)
                    nc.tensor.matmul(
                        spA[:, qc, :], kpt[d][0:DK, ksl], qpt[d][0:DK, qsl],
                        start=True, stop=True,
                    )
                for qc in range(2):
                    qsl = slice(qc * QC, (qc + 1) * QC)
                    nc.tensor.matmul(
                        spB[:, qc, :], kpt[d][DK:P, ksl], qpt[d][DK:P, qsl],
                        start=True, stop=True,
                    )
                nc.scalar.activation(
                    ptA[:, kb, :, :], spA, AF.Exp, scale=0.03125
                )
                nc.scalar.activation(
                    ptB[:, kb, :, :], spB, AF.Exp, scale=0.03125
                )
                if pv_gen is not None:
                    next(pv_gen, None)
            return ptA, ptB

        # ---- V ----------------------------------------------------------
        with nc.named_scope("v_prep"):
            vt = load_transposed(Vd)
            wv = wload(WVd)
            wo = wload(WOd)
            vaug = [
                sb.tile([P, H * 65], bf16, tag="vaug", bufs=NB, name=f"vaug{i}")
                for i in range(NB)
            ]
            for sblk in range(NB):
                nc.vector.memset(
                    vaug[sblk].rearrange("p (h c) -> p h c", c=65)[:, :, 64:65],
                    1.0,
                )

        def gen_vproj():
            """V projection, one seq-block per step — interleaved into the
            hoisted unit-0 scores so the PE keeps streaming while unit-0's
            exps drain the scores PSUM ring."""
            for sblk in range(NB):
                acc = [
                    ps.tile([P, QC], f32, tag="vp", bufs=2, name="vacc")
                    for _ in range(2)
                ]
                if with_bv:
                    for c in range(2):
                        nc.tensor.matmul(
                            acc[c],
                            onesb[0:1, 0:P],
                            bv_row[0:1, c * QC : (c + 1) * QC],
                            start=True, stop=False,
                        )
                for kb in range(NB):
                    for c in range(2):
                        nc.tensor.matmul(
                            acc[c],
                            vt[kb][:, sblk * P : (sblk + 1) * P],
                            wv[:, kb, c * QC : (c + 1) * QC],
                            start=(kb == 0 and not with_bv),
                            stop=(kb == NB - 1),
                        )
                for c in range(2):
                    dst = vaug[sblk].rearrange("p (h c) -> p h c", c=65)[
                        :, c * 8 : (c + 1) * 8, 0:64
                    ]
                    nc.vector.tensor_scalar_max(
                        dst, acc[c].rearrange("p (h c) -> p h c", c=64), 0.0
                    )
                yield

        with nc.named_scope("v_proj"):
            vg = gen_vproj()
            early_unit = emit_scores_unit(0, vg)
            for _ in vg:
                pass

        # ---- attention --------------------------------------------------
        ot = [
            sb.tile([P, S], bf16, tag="ot", bufs=NB, name=f"ot{i}")
            for i in range(NB)
        ]

        def emit_pv_tail(h, vp):
            """Softmax division: spread denom row -> DVE reciprocal ->
            casting gather -> K=1 PE partition-broadcast -> DVE multiply."""
            dbq, off = h // 2, (h % 2) * DK
            for qc in range(2):
                qsl = slice(qc * QC, (qc + 1) * QC)
                stage = sb.tile([65, QC], f32, tag="stage", bufs=4, name="stage")
                nc.vector.tensor_copy(stage, vp[qc][0:65, :])
                # SBUF can't scatter one partition's row across partitions
                # (illegal partition step), so bounce the denom row off DRAM
                scr = dramp.tile([1, QC], f32, tag="scr", bufs=4, name="scr")
                nc.sync.dma_start(scr, stage[64:65, :])
                rsp = sb.tile([DK, NB], f32, tag="rsp", bufs=4, name="rsp")
                nc.sync.dma_start(
                    rsp, scr.rearrange("o (a b) -> a (o b)", a=DK)
                )
                nc.vector.reciprocal(rsp, rsp)
                rrow = sb.tile([1, QC], bf16, tag="rrow", bufs=4, name="rrow")
                # [64, 8] -> [1, 512] partition-major gather (+ f32->bf16
                # cast, hence gpsimd); AP balancing maps (a, b) -> a*8+b
                nc.gpsimd.dma_start(rrow.rearrange("o (a b) -> o a b", a=DK), rsp)
                bc = ps.tile([DK, QC], f32, tag="bc", bufs=2, name="bc")
                nc.tensor.matmul(bc, onesb[0:1, 0:DK], rrow, start=True, stop=True)
                if off == 0:
                    nc.vector.tensor_mul(ot[dbq][0:DK, qsl], stage[0:DK, :], bc)
                else:
                    tmp = sb.tile([DK, QC], bf16, tag="tmp", bufs=2, name="tmp")
                    nc.vector.tensor_mul(tmp, stage[0:DK, :], bc)
                    nc.gpsimd.dma_start(ot[dbq][DK:P, qsl], tmp)

        def gen_pv_pair(d, ptA, ptB):
            """PV + softmax division for head pair (2d, 2d+1), both q-chunks,
            yielded in 8 groups of 4 matmuls for interleaving."""
            for hl, ptX in ((0, ptA), (1, ptB)):
                h = 2 * d + hl
                vp = [
                    ps.tile([P, QC], f32, tag="vp", bufs=2, name="vpacc")
                    for _ in range(2)
                ]
                for g in range(4):
                    for kb in (2 * g, 2 * g + 1):
                        for qc in range(2):
                            nc.tensor.matmul(
                                vp[qc][0:65, :],
                                vaug[kb][:, h * 65 : (h + 1) * 65],
                                ptX[:, kb, qc, :],
                                start=(kb == 0),
                                stop=(kb == NB - 1),
                            )
                    yield
                emit_pv_tail(h, vp)

        def emit_oproj():
            """O-projection with contraction block 7 deferred: blocks 0-6 of
            all seq-blocks have no dependency on the last head pair, so the
            PE starts them while the last softmax divisions drain on Vector;
            block 7 + eviction land per seq-block as the PSUM ring allows."""
            accs = [None] * NB

            def finish(s):
                bigacc = accs[s]
                for c in range(2):
                    nc.tensor.matmul(
                        bigacc[:, c, :],
                        ot[NB - 1][:, s * P : (s + 1) * P],
                        wo[:, NB - 1, c * QC : (c + 1) * QC],
                        start=False, stop=True,
                    )
                for c in range(2):
                    o = sb.tile([P, QC], f32, tag="obuf", bufs=3, name="obuf")
                    nc.vector.tensor_scalar_max(o, bigacc[:, c, :], 0.0)
                    nc.sync.dma_start(
                        outd[s * P : (s + 1) * P, c * QC : (c + 1) * QC], o
                    )
                accs[s] = None

            for s in range(NB):
                bigacc = ps.tile([P, 2, QC], f32, tag="big", bufs=2, name="oacc")
                accs[s] = bigacc
                if with_bo:
                    for c in range(2):
                        nc.tensor.matmul(
                            bigacc[:, c, :],
                            onesb[0:1, 0:P],
                            bo_row[0:1, c * QC : (c + 1) * QC],
                            start=True, stop=False,
                        )
                for db in range(NB - 1):
                    for c in range(2):
                        nc.tensor.matmul(
                            bigacc[:, c, :],
                            ot[db][:, s * P : (s + 1) * P],
                            wo[:, db, c * QC : (c + 1) * QC],
                            start=(db == 0 and not with_bo),
                            stop=False,
                        )
                if s >= 1:
                    finish(s - 1)
            finish(NB - 1)

        with nc.named_scope("attention"):
            # software-pipelined over head pairs; the previous pair's PV
            # matmul groups ride inside the current scores unit. Pair 0's
            # scores were hoisted before the V phase.
            pend = early_unit
            for d in range(1, NB):
                g = gen_pv_pair(d - 1, *pend)
                pend = emit_scores_unit(d, g)
                for _ in g:
                    pass

        # ---- output projection --------------------------------------
        with nc.named_scope("o_proj"):
            for _ in gen_pv_pair(NB - 1, *pend):
                pass
            emit_oproj()

    _split_wide_waits(nc)
    return nc


_NC_CACHE = {}


def kernel(Q, K, V, WQ, bQ, WK, bK, WV, bV, WO, bO, h):
    bfl = ml_dtypes.bfloat16
    Q, K, V = (np.ascontiguousarray(np.asarray(x, np.float32).astype(bfl))
               for x in (Q, K, V))
    WQ, WK, WV, WO = (
        np.ascontiguousarray(np.asarray(x, np.float32).astype(bfl))
        for x in (WQ, WK, WV, WO)
    )
    bQ, bK, bV, bO = (
        np.ascontiguousarray(np.asarray(x, np.float32)) for x in (bQ, bK, bV, bO)
    )
    h = int(np.asarray(h))
    assert h == H, f"kernel specialized for h=16, got {h}"
    B = Q.shape[0]
    assert Q.shape == (B, S, D) and B == N_CORES

    key = (bool(np.any(bV)), bool(np.any(bO)))
    if key not in _NC_CACHE:
        _NC_CACHE[key] = _build_nc(*key)
    nc = _NC_CACHE[key]

    in_maps = [
        {
            "Q": Q[b], "K": K[b], "V": V[b],
            "WQ": WQ, "WK": WK, "WV": WV, "WO": WO,
            "bQ": bQ, "bK": bK, "bV": bV, "bO": bO,
        }
        for b in range(B)
    ]
    trace = os.environ.get("BASS_MHA_TRACE") == "1"
    res = run_bass_kernel_spmd(
        nc, in_maps, core_ids=list(range(N_CORES)), trace=trace
    )
    if trace:
        kernel.last_results = res
    return np.stack([res.results[b]["out"] for b in range(B)], axis=0)
